# revision 15
# baseline (speedup 1.0000x reference)
"""Trainium2 Bass kernel for nn_ConvAttention.

The reference computes:
    fx = conv1x1(x, wf) + bf          # [B,1,H,W]
    gx = conv1x1(x, wg) + bg
    hx = conv1x1(x, wh) + bh
    a  = softmax(fx @ gx, axis=1)     # axis of size 1 -> identically 1.0
    o  = (hx @ a) * x                 # hx @ ones = row-sum broadcast over W

Because the softmax is over a size-1 axis it is exactly 1.0 everywhere, so
    o[b,c,i,j] = s[b,i] * x[b,c,i,j]
    s[b,i]     = sum_c sum_k x[b,c,i,k] * wh[c] + W * bh
wf/bf/wg/bg do not affect the output. The kernel streams x once through
SBUF - purely memory bound. Sharding: pure data parallel over batch;
4 batches per core on 8 cores, weights replicated.

v3 pipeline (HW-measured evolution; see git-style history in test logs):
  * Loads are SWDGE (gpsimd) DMAs that cast fp32 -> bf16 in the DMA
    datapath (HW-measured: same rate as plain HWDGE loads). HBM read
    traffic is unchanged (16 MiB f32/core) but SBUF holds bf16, and the
    PE can then run 1-pass bf16 matmuls (fp32 matmuls are 2-pass HI/LO
    and were the v1 bottleneck at 88 us/core).
  * The channel contraction runs on the PE with a replicated stationary
    operand: wh_bcast[p, j*128+m] = wh[j*128+p] for all m, so
    psum[m, (i,k)] = sum_c wh[c] x[c,i,k] lands broadcast across all 128
    partitions. One accumulating matmul pair per 512-column block.
  * s = row-sum(psum) + W*bh: full-lane DVE reduce (PSUM -> SBUF) then a
    tiny ACT bias add. This replaces v2's 35 us of SBUF row-sum reduces
    with 17 us of PSUM reduces (the j-chunk dim is pre-contracted).
  * o = s * x on DVE (bf16 in, fp16 out; two mid groups on the Pool
    engine to shave DVE's peak). Output is stored as fp16 and widened to
    fp32 on the host: halves HBM write traffic (8 MiB/core), max rel err
    ~5e-3 << the 2e-2 gate.
  * The last batch is loaded and computed in tapering groups
    (24/24/8/8 rows) so the post-last-load drain chain is short.

DMA roofline for this traffic (25.2 MB/core, HW-measured with no
compute): 74.4 us end-to-end incl the ~7 us Tile preamble.
"""

from contextlib import ExitStack

import numpy as np

B, C, H, W = 32, 256, 64, 64
N_CORES = 8
BS = B // N_CORES  # batches per core

_CACHE = {}


def _split_multi_waits(nc, mybir):
    """Walrus codegen allows only one sync-wait slot on most instruction
    encodings ("Too many sync wait commands"). Tile's sem assigner sometimes
    attaches 2-3. Hoist the extras onto standalone EventSemaphore
    instructions immediately before, on the same engine - semantically
    identical since engines execute their stream in order."""
    n = 0
    for f in nc.m.functions:
        for bb in f.blocks:
            new_insts = []
            for inst in bb.instructions:
                si = inst.sync_info
                ow = list(si.on_wait) if si and si.on_wait else []
                if len(ow) > 1:
                    for wv in ow[:-1]:
                        n += 1
                        evs = mybir.InstEventSemaphore(
                            name=f"evs_split_{n}",
                            ins=[],
                            outs=[],
                            engine=inst.engine,
                            bass_nofuse=True,
                            sync_info=mybir.SyncInfo(on_wait=[wv], on_update=[]),
                        )
                        nc.register_instruction(evs, overwrite=True)
                        new_insts.append(evs)
                    inst.sync_info = mybir.SyncInfo(
                        on_wait=[ow[-1]],
                        on_update=list(si.on_update) if si.on_update else [],
                    )
                new_insts.append(inst)
            bb.instructions = new_insts
    return n


def _build(bs, c, h, w):
    import concourse.bass as bass
    import concourse.tile as tile
    from concourse import mybir

    f32 = mybir.dt.float32
    f16 = mybir.dt.float16
    bf16 = mybir.dt.bfloat16
    P = 128
    n_ch = c // P
    assert n_ch == 2 and c % P == 0
    hw = h * w

    nc = bass.Bass("TRN2", target_bir_lowering=False, debug=False)
    x = nc.dram_tensor("x", [bs, c, h, w], f32, kind="ExternalInput").ap()
    wh = nc.dram_tensor("wh", [c], f32, kind="ExternalInput").ap()
    bh = nc.dram_tensor("bh", [1], f32, kind="ExternalInput").ap()
    o = nc.dram_tensor("o", [bs, c, h, w], f16, kind="ExternalOutput").ap()

    X = mybir.AxisListType.X

    # Load pieces (per c-chunk) and compute groups (h-row ranges), with a
    # tapered tail on the last batch so the final drain chain is short.
    if h == 64 and bs >= 2:
        load_rows = {b: [(0, h)] for b in range(bs - 1)}
        load_rows[bs - 1] = [(0, 48), (48, 16)]
        grp_rows = {b: [(0, 32), (32, 32)] for b in range(bs - 1)}
        grp_rows[bs - 1] = [(0, 24), (24, 24), (48, 8), (56, 8)]
    else:
        hh = h // 2 if h % 2 == 0 else h
        load_rows = {b: [(0, h)] for b in range(bs)}
        grp_rows = {
            b: [(i * hh, hh) for i in range(h // hh)] for b in range(bs)
        }
    groups = [(b, h0, hr) for b in range(bs) for (h0, hr) in grp_rows[b]]
    # Multiply engine: DVE everywhere except three mid groups on Pool
    # (Pool is ~1.7x slower per element but otherwise idle after load
    # issue; DVE carries the PSUM reduces and was the busiest engine).
    pool_mult = {g for g in range(len(groups)) if g in (3, 4, 5)}

    with tile.TileContext(nc) as tc, ExitStack() as ctx:
        consts = ctx.enter_context(tc.tile_pool(name="consts", bufs=1))
        xpool = ctx.enter_context(tc.tile_pool(name="xp", bufs=bs))
        opool = ctx.enter_context(tc.tile_pool(name="op", bufs=4))
        spool = ctx.enter_context(tc.tile_pool(name="sp", bufs=4))
        rpool = ctx.enter_context(tc.tile_pool(name="rp", bufs=4))
        psp = ctx.enter_context(tc.tile_pool(name="ps", bufs=4, space="PSUM"))

        # wh as bf16 [128, n_ch*128]: wh_bcast[p, j*128+m] = wh[j*128+p]
        # for all m - the contraction matmul then broadcasts s to all
        # 128 output partitions for free.
        wh_raw = consts.tile([P, n_ch], f32)
        nc.sync.dma_start(wh_raw[:], wh.rearrange("(j p) -> p j", p=P))
        wh_bcast = consts.tile([P, n_ch * P], bf16)
        nc.vector.tensor_copy(
            wh_bcast[:].rearrange("p (j m) -> p j m", j=n_ch),
            wh_raw[:, :, None].broadcast_to((P, n_ch, P)),
        )
        # W*bh replicated on all partitions, for the final bias add
        bh_sb = consts.tile([P, 1], f32)
        nc.sync.dma_start(bh_sb[:], bh.to_broadcast((P, 1)))
        biasW = consts.tile([P, 1], f32)
        nc.scalar.mul(biasW[:], bh_sb[:], float(w))

        # All loads first: SWDGE fp32 -> bf16 cast DMAs on the Pool queue.
        # (Pool's multiply work is emitted after every load is issued.)
        xbs = {}
        for b in range(bs):
            xb = xpool.tile([P, n_ch * hw], bf16)
            xbs[b] = xb
            for ch in range(n_ch):
                for r0, rn in load_rows[b]:
                    nc.gpsimd.dma_start(
                        xb[:, ch * hw + r0 * w : ch * hw + (r0 + rn) * w],
                        x[b, ch * P : (ch + 1) * P, r0 : r0 + rn].rearrange(
                            "c h w -> c (h w)"
                        ),
                    )

        # How many h-rows fit a 512-column matmul block
        rb = max(1, min(512 // w, h))

        for g, (b, h0, hr) in enumerate(groups):
            xb = xbs[b]
            xg = xb[:].rearrange("c (j h w) -> c j h w", j=n_ch, h=h)[
                :, :, h0 : h0 + hr
            ]

            # hx broadcast to all partitions: psum[m, (i,k)] = sum_c
            # wh[c] x[c, h0+i, k], accumulated over the two c-chunks,
            # one matmul pair per rb-row (512-col) block. Two psum tiles
            # per group (2 banks each) so the DVE reduce of the first
            # half overlaps the matmuls of the second - the PE is HAM
            # clock-throttled (K=4/8 for most of the run) so matmul
            # latency is the long pole of the per-group chain.
            rsg = rpool.tile([P, hr], f32)
            hs = max(rb, hr // 2)
            for p0 in range(0, hr, hs):
                pn = min(hs, hr - p0)
                pt = psp.tile([P, pn * w], f32)
                for q in range(0, pn, rb):
                    qn = min(rb, pn - q)
                    reg = pt[:, q * w : (q + qn) * w]
                    for ch in range(n_ch):
                        nc.tensor.matmul(
                            reg,
                            lhsT=wh_bcast[:, ch * P : (ch + 1) * P],
                            rhs=xg[:, ch, p0 + q : p0 + q + qn].rearrange(
                                "c h w -> c (h w)"
                            ),
                            start=(ch == 0),
                            stop=(ch == n_ch - 1),
                        )
                # s = row-sum(hx): full-lane PSUM -> SBUF reduce
                nc.vector.reduce_sum(
                    rsg[:, p0 : p0 + pn],
                    pt[:].rearrange("p (h w) -> p h w", w=w),
                    axis=X,
                )
            s128 = spool.tile([P, hr], f32)
            nc.scalar.add(s128[:], rsg[:], biasW[:])

            # o = s * x quantized to fp16, then store this group's rows
            ot = opool.tile([P, n_ch * hr * w], f16)
            eng = nc.gpsimd if g in pool_mult else nc.vector
            eng.tensor_mul(
                ot[:].rearrange("c (j h w) -> c j h w", j=n_ch, h=hr),
                xg,
                s128[:, None, :, None].broadcast_to((P, n_ch, hr, w)),
            )
            nc.sync.dma_start(
                o[b, :, h0 : h0 + hr].rearrange("(j c) h w -> c j h w", c=P),
                ot[:].rearrange("c (j h w) -> c j h w", j=n_ch, h=hr),
            )
    _split_multi_waits(nc, mybir)
    return nc


def get_nc(bs=BS, c=C, h=H, w=W):
    key = (bs, c, h, w)
    if key not in _CACHE:
        _CACHE[key] = _build(bs, c, h, w)
    return _CACHE[key]


def kernel(x, wf, bf, wg, bg, wh, bh, **_unused):
    from concourse.bass_utils import run_bass_kernel_spmd

    x = np.ascontiguousarray(np.asarray(x, dtype=np.float32))
    wh = np.ascontiguousarray(np.asarray(wh, dtype=np.float32))
    bh = np.ascontiguousarray(np.asarray(bh, dtype=np.float32))

    in_maps = [
        {"x": x[k * BS : (k + 1) * BS], "wh": wh, "bh": bh} for k in range(N_CORES)
    ]
    # Tile scheduling is nondeterministic build-to-build and a rare schedule
    # can deadlock on hardware (NRT unrecoverable). Rebuilding produces a
    # fresh schedule, so retry with a clean build on any execution failure.
    last_err = None
    for attempt in range(3):
        try:
            nc = get_nc()
            res = run_bass_kernel_spmd(nc, in_maps, core_ids=list(range(N_CORES)))
            return np.concatenate(
                [
                    np.asarray(res.results[k]["o"], dtype=np.float32)
                    for k in range(N_CORES)
                ],
                axis=0,
            )
        except Exception as e:  # rebuild with a new schedule and retry
            last_err = e
            _CACHE.clear()
    raise last_err


# revision 17
# speedup vs baseline: 1.0749x; 1.0749x over previous
"""Trainium2 Bass kernel for nn_ConvAttention.

The reference computes:
    fx = conv1x1(x, wf) + bf          # [B,1,H,W]
    gx = conv1x1(x, wg) + bg
    hx = conv1x1(x, wh) + bh
    a  = softmax(fx @ gx, axis=1)     # axis of size 1 -> identically 1.0
    o  = (hx @ a) * x                 # hx @ ones = row-sum broadcast over W

Because the softmax is over a size-1 axis it is exactly 1.0 everywhere, so
    o[b,c,i,j] = s[b,i] * x[b,c,i,j]
    s[b,i]     = sum_c sum_k x[b,c,i,k] * wh[c] + W * bh
wf/bf/wg/bg do not affect the output. The kernel streams x once through
SBUF - purely memory bound. Sharding: pure data parallel over batch;
4 batches per core on 8 cores, weights replicated.

v3 pipeline (HW-measured evolution; see git-style history in test logs):
  * Loads are SWDGE (gpsimd) DMAs that cast fp32 -> bf16 in the DMA
    datapath (HW-measured: same rate as plain HWDGE loads). HBM read
    traffic is unchanged (16 MiB f32/core) but SBUF holds bf16, and the
    PE can then run 1-pass bf16 matmuls (fp32 matmuls are 2-pass HI/LO
    and were the v1 bottleneck at 88 us/core).
  * The channel contraction runs on the PE with a replicated stationary
    operand: wh_bcast[p, j*128+m] = wh[j*128+p] for all m, so
    psum[m, (i,k)] = sum_c wh[c] x[c,i,k] lands broadcast across all 128
    partitions. One accumulating matmul pair per 512-column block.
  * s = row-sum(psum) + W*bh: full-lane DVE reduce (PSUM -> SBUF) then a
    tiny ACT bias add. This replaces v2's 35 us of SBUF row-sum reduces
    with 17 us of PSUM reduces (the j-chunk dim is pre-contracted).
  * o = s * x on DVE (bf16 in, fp16 out; two mid groups on the Pool
    engine to shave DVE's peak). Output is stored as fp16 and widened to
    fp32 on the host: halves HBM write traffic (8 MiB/core), max rel err
    ~5e-3 << the 2e-2 gate.
  * The last batch is loaded and computed in tapering groups
    (24/24/8/8 rows) so the post-last-load drain chain is short.

DMA roofline for this traffic (25.2 MB/core, HW-measured with no
compute): 74.4 us end-to-end incl the ~7 us Tile preamble.
"""

from contextlib import ExitStack

import numpy as np

B, C, H, W = 32, 256, 64, 64
N_CORES = 8
BS = B // N_CORES  # batches per core

_CACHE = {}


def _split_multi_waits(nc, mybir):
    """Walrus codegen allows only one sync-wait slot on most instruction
    encodings ("Too many sync wait commands"). Tile's sem assigner sometimes
    attaches 2-3. Hoist the extras onto standalone EventSemaphore
    instructions immediately before, on the same engine - semantically
    identical since engines execute their stream in order."""
    n = 0
    for f in nc.m.functions:
        for bb in f.blocks:
            new_insts = []
            for inst in bb.instructions:
                si = inst.sync_info
                ow = list(si.on_wait) if si and si.on_wait else []
                if len(ow) > 1:
                    for wv in ow[:-1]:
                        n += 1
                        evs = mybir.InstEventSemaphore(
                            name=f"evs_split_{n}",
                            ins=[],
                            outs=[],
                            engine=inst.engine,
                            bass_nofuse=True,
                            sync_info=mybir.SyncInfo(on_wait=[wv], on_update=[]),
                        )
                        nc.register_instruction(evs, overwrite=True)
                        new_insts.append(evs)
                    inst.sync_info = mybir.SyncInfo(
                        on_wait=[ow[-1]],
                        on_update=list(si.on_update) if si.on_update else [],
                    )
                new_insts.append(inst)
            bb.instructions = new_insts
    return n


def _build(bs, c, h, w):
    import concourse.bass as bass
    import concourse.tile as tile
    from concourse import mybir

    f32 = mybir.dt.float32
    f16 = mybir.dt.float16
    bf16 = mybir.dt.bfloat16
    P = 128
    n_ch = c // P
    assert n_ch == 2 and c % P == 0
    hw = h * w

    nc = bass.Bass("TRN2", target_bir_lowering=False, debug=False)
    x = nc.dram_tensor("x", [bs, c, h, w], f32, kind="ExternalInput").ap()
    wh = nc.dram_tensor("wh", [c], f32, kind="ExternalInput").ap()
    bh = nc.dram_tensor("bh", [1], f32, kind="ExternalInput").ap()
    o = nc.dram_tensor("o", [bs, c, h, w], f16, kind="ExternalOutput").ap()

    X = mybir.AxisListType.X

    # Load pieces (per c-chunk) and compute groups (h-row ranges), with a
    # tapered tail on the last batch so the final drain chain is short.
    if h == 64 and bs >= 2:
        load_rows = {b: [(0, h)] for b in range(bs - 1)}
        load_rows[bs - 1] = [(0, 48), (48, 16)]
        grp_rows = {b: [(0, 32), (32, 32)] for b in range(bs - 1)}
        grp_rows[bs - 1] = [(0, 24), (24, 24), (48, 8), (56, 8)]
    else:
        hh = h // 2 if h % 2 == 0 else h
        load_rows = {b: [(0, h)] for b in range(bs)}
        grp_rows = {
            b: [(i * hh, hh) for i in range(h // hh)] for b in range(bs)
        }
    groups = [(b, h0, hr) for b in range(bs) for (h0, hr) in grp_rows[b]]
    # Multiply engine: DVE everywhere except two early-mid groups on Pool.
    # Pool multiplies are ~2.2x slower (incl. Q7 drains) and execute
    # in-order on the same engine that issues the SWDGE loads, so they are
    # only safe for groups whose s is ready while loads still stream;
    # back-to-back late Pool multiplies serialize the drain (HW-measured
    # +6 us with groups {3,4,5} on Pool).
    pool_mult = {g for g in range(len(groups)) if g in (2, 4)}

    with tile.TileContext(nc) as tc, ExitStack() as ctx:
        consts = ctx.enter_context(tc.tile_pool(name="consts", bufs=1))
        xpool = ctx.enter_context(tc.tile_pool(name="xp", bufs=bs))
        opool = ctx.enter_context(tc.tile_pool(name="op", bufs=4))
        spool = ctx.enter_context(tc.tile_pool(name="sp", bufs=4))
        rpool = ctx.enter_context(tc.tile_pool(name="rp", bufs=4))
        psp = ctx.enter_context(tc.tile_pool(name="ps", bufs=4, space="PSUM"))

        # wh as bf16 [128, n_ch*128]: wh_bcast[p, j*128+m] = wh[j*128+p]
        # for all m - the contraction matmul then broadcasts s to all
        # 128 output partitions for free.
        wh_raw = consts.tile([P, n_ch], f32)
        nc.sync.dma_start(wh_raw[:], wh.rearrange("(j p) -> p j", p=P))
        wh_bcast = consts.tile([P, n_ch * P], bf16)
        nc.vector.tensor_copy(
            wh_bcast[:].rearrange("p (j m) -> p j m", j=n_ch),
            wh_raw[:, :, None].broadcast_to((P, n_ch, P)),
        )
        # W*bh replicated on all partitions, for the final bias add
        bh_sb = consts.tile([P, 1], f32)
        nc.sync.dma_start(bh_sb[:], bh.to_broadcast((P, 1)))
        biasW = consts.tile([P, 1], f32)
        nc.scalar.mul(biasW[:], bh_sb[:], float(w))

        # All loads first: SWDGE fp32 -> bf16 cast DMAs on the Pool queue.
        # (Pool's multiply work is emitted after every load is issued.)
        xbs = {}
        for b in range(bs):
            xb = xpool.tile([P, n_ch * hw], bf16)
            xbs[b] = xb
            for ch in range(n_ch):
                for r0, rn in load_rows[b]:
                    nc.gpsimd.dma_start(
                        xb[:, ch * hw + r0 * w : ch * hw + (r0 + rn) * w],
                        x[b, ch * P : (ch + 1) * P, r0 : r0 + rn].rearrange(
                            "c h w -> c (h w)"
                        ),
                    )

        # How many h-rows fit a 512-column matmul block
        rb = max(1, min(512 // w, h))

        for g, (b, h0, hr) in enumerate(groups):
            xb = xbs[b]
            xg = xb[:].rearrange("c (j h w) -> c j h w", j=n_ch, h=h)[
                :, :, h0 : h0 + hr
            ]

            # hx broadcast to all partitions: psum[m, (i,k)] = sum_c
            # wh[c] x[c, h0+i, k], accumulated over the two c-chunks,
            # one matmul pair per rb-row (512-col) block. Two psum tiles
            # per group (2 banks each) so the DVE reduce of the first
            # half overlaps the matmuls of the second - the PE is HAM
            # clock-throttled (K=4/8 for most of the run) so matmul
            # latency is the long pole of the per-group chain.
            rsg = rpool.tile([P, hr], f32)
            hs = max(rb, hr // 2)
            for p0 in range(0, hr, hs):
                pn = min(hs, hr - p0)
                pt = psp.tile([P, pn * w], f32)
                for q in range(0, pn, rb):
                    qn = min(rb, pn - q)
                    reg = pt[:, q * w : (q + qn) * w]
                    for ch in range(n_ch):
                        nc.tensor.matmul(
                            reg,
                            lhsT=wh_bcast[:, ch * P : (ch + 1) * P],
                            rhs=xg[:, ch, p0 + q : p0 + q + qn].rearrange(
                                "c h w -> c (h w)"
                            ),
                            start=(ch == 0),
                            stop=(ch == n_ch - 1),
                        )
                # s = row-sum(hx): full-lane PSUM -> SBUF reduce
                nc.vector.reduce_sum(
                    rsg[:, p0 : p0 + pn],
                    pt[:].rearrange("p (h w) -> p h w", w=w),
                    axis=X,
                )
            s128 = spool.tile([P, hr], f32)
            nc.scalar.add(s128[:], rsg[:], biasW[:])

            # o = s * x quantized to fp16, then store this group's rows
            ot = opool.tile([P, n_ch * hr * w], f16)
            eng = nc.gpsimd if g in pool_mult else nc.vector
            eng.tensor_mul(
                ot[:].rearrange("c (j h w) -> c j h w", j=n_ch, h=hr),
                xg,
                s128[:, None, :, None].broadcast_to((P, n_ch, hr, w)),
            )
            nc.scalar.dma_start(
                o[b, :, h0 : h0 + hr].rearrange("(j c) h w -> c j h w", c=P),
                ot[:].rearrange("c (j h w) -> c j h w", j=n_ch, h=hr),
            )
    _split_multi_waits(nc, mybir)
    return nc


def get_nc(bs=BS, c=C, h=H, w=W):
    key = (bs, c, h, w)
    if key not in _CACHE:
        _CACHE[key] = _build(bs, c, h, w)
    return _CACHE[key]


def kernel(x, wf, bf, wg, bg, wh, bh, **_unused):
    from concourse.bass_utils import run_bass_kernel_spmd

    x = np.ascontiguousarray(np.asarray(x, dtype=np.float32))
    wh = np.ascontiguousarray(np.asarray(wh, dtype=np.float32))
    bh = np.ascontiguousarray(np.asarray(bh, dtype=np.float32))

    in_maps = [
        {"x": x[k * BS : (k + 1) * BS], "wh": wh, "bh": bh} for k in range(N_CORES)
    ]
    # Tile scheduling is nondeterministic build-to-build and a rare schedule
    # can deadlock on hardware (NRT unrecoverable). Rebuilding produces a
    # fresh schedule, so retry with a clean build on any execution failure.
    last_err = None
    for attempt in range(3):
        try:
            nc = get_nc()
            res = run_bass_kernel_spmd(nc, in_maps, core_ids=list(range(N_CORES)))
            return np.concatenate(
                [
                    np.asarray(res.results[k]["o"], dtype=np.float32)
                    for k in range(N_CORES)
                ],
                axis=0,
            )
        except Exception as e:  # rebuild with a new schedule and retry
            last_err = e
            _CACHE.clear()
    raise last_err


# revision 21
# speedup vs baseline: 1.0774x; 1.0024x over previous
"""Trainium2 Bass kernel for nn_ConvAttention.

The reference computes:
    fx = conv1x1(x, wf) + bf          # [B,1,H,W]
    gx = conv1x1(x, wg) + bg
    hx = conv1x1(x, wh) + bh
    a  = softmax(fx @ gx, axis=1)     # axis of size 1 -> identically 1.0
    o  = (hx @ a) * x                 # hx @ ones = row-sum broadcast over W

Because the softmax is over a size-1 axis it is exactly 1.0 everywhere, so
    o[b,c,i,j] = s[b,i] * x[b,c,i,j]
    s[b,i]     = sum_c sum_k x[b,c,i,k] * wh[c] + W * bh
wf/bf/wg/bg do not affect the output. The kernel streams x once through
SBUF - purely memory bound. Sharding: pure data parallel over batch;
4 batches per core on 8 cores, weights replicated.

v3 pipeline (HW-measured evolution; see git-style history in test logs):
  * Loads are SWDGE (gpsimd) DMAs that cast fp32 -> bf16 in the DMA
    datapath (HW-measured: same rate as plain HWDGE loads). HBM read
    traffic is unchanged (16 MiB f32/core) but SBUF holds bf16, and the
    PE can then run 1-pass bf16 matmuls (fp32 matmuls are 2-pass HI/LO
    and were the v1 bottleneck at 88 us/core).
  * The channel contraction runs on the PE with a replicated stationary
    operand: wh_bcast[p, j*128+m] = wh[j*128+p] for all m, so
    psum[m, (i,k)] = sum_c wh[c] x[c,i,k] lands broadcast across all 128
    partitions. One accumulating matmul pair per 512-column block.
  * s = row-sum(psum) + W*bh: full-lane DVE reduce (PSUM -> SBUF) then a
    tiny ACT bias add. This replaces v2's 35 us of SBUF row-sum reduces
    with 17 us of PSUM reduces (the j-chunk dim is pre-contracted).
  * o = s * x on DVE (bf16 in, fp16 out; two mid groups on the Pool
    engine to shave DVE's peak). Output is stored as fp16 and widened to
    fp32 on the host: halves HBM write traffic (8 MiB/core), max rel err
    ~5e-3 << the 2e-2 gate.
  * The last batch is loaded and computed in tapering groups
    (24/24/8/8 rows) so the post-last-load drain chain is short.

DMA roofline for this traffic (25.2 MB/core, HW-measured with no
compute): 74.4 us end-to-end incl the ~7 us Tile preamble.
"""

from contextlib import ExitStack

import numpy as np

B, C, H, W = 32, 256, 64, 64
N_CORES = 8
BS = B // N_CORES  # batches per core

_CACHE = {}


def _split_multi_waits(nc, mybir):
    """Walrus codegen allows only one sync-wait slot on most instruction
    encodings ("Too many sync wait commands"). Tile's sem assigner sometimes
    attaches 2-3. Hoist the extras onto standalone EventSemaphore
    instructions immediately before, on the same engine - semantically
    identical since engines execute their stream in order."""
    n = 0
    for f in nc.m.functions:
        for bb in f.blocks:
            new_insts = []
            for inst in bb.instructions:
                si = inst.sync_info
                ow = list(si.on_wait) if si and si.on_wait else []
                if len(ow) > 1:
                    for wv in ow[:-1]:
                        n += 1
                        evs = mybir.InstEventSemaphore(
                            name=f"evs_split_{n}",
                            ins=[],
                            outs=[],
                            engine=inst.engine,
                            bass_nofuse=True,
                            sync_info=mybir.SyncInfo(on_wait=[wv], on_update=[]),
                        )
                        nc.register_instruction(evs, overwrite=True)
                        new_insts.append(evs)
                    inst.sync_info = mybir.SyncInfo(
                        on_wait=[ow[-1]],
                        on_update=list(si.on_update) if si.on_update else [],
                    )
                new_insts.append(inst)
            bb.instructions = new_insts
    return n


def _build(bs, c, h, w):
    import concourse.bass as bass
    import concourse.tile as tile
    from concourse import mybir

    f32 = mybir.dt.float32
    f16 = mybir.dt.float16
    bf16 = mybir.dt.bfloat16
    P = 128
    n_ch = c // P
    assert n_ch == 2 and c % P == 0
    hw = h * w

    nc = bass.Bass("TRN2", target_bir_lowering=False, debug=False)
    x = nc.dram_tensor("x", [bs, c, h, w], f32, kind="ExternalInput").ap()
    wh = nc.dram_tensor("wh", [c], f32, kind="ExternalInput").ap()
    bh = nc.dram_tensor("bh", [1], f32, kind="ExternalInput").ap()
    o = nc.dram_tensor("o", [bs, c, h, w], f16, kind="ExternalOutput").ap()

    X = mybir.AxisListType.X

    # Load pieces (b, c-chunk, row range) in issue order, and compute
    # groups (b, h0, hr) in emission order. The last-arriving HBM bytes
    # gate the drain chain, so the final pieces are two 8-row slivers of
    # the last batch (HW-measured: with whole batches loaded in order,
    # the entire last batch's ~17 us of reduce+multiply+store work sat
    # after the final load). The last batch's first 48 rows load FIRST
    # and are processed early instead.
    lb = bs - 1
    if h == 64 and bs >= 2:
        load_pieces = (
            [(lb, 0, 48)]
            + [(b, 0, h) for b in range(bs - 1)]
            + [(lb, 48, 8), (lb, 56, 8)]
        )
        groups = (
            [(lb, 0, 24), (lb, 24, 24)]
            + [(b, h0, 32) for b in range(bs - 1) for h0 in (0, 32)]
            + [(lb, 48, 8), (lb, 56, 8)]
        )
        # Multiply engine: DVE everywhere except spaced early-mid groups
        # on Pool. Pool multiplies are ~2.2x slower (incl. Q7 drains) and
        # run in-order on the engine that issues the SWDGE loads, so they
        # are only safe for groups whose s is ready while loads still
        # stream; late back-to-back Pool multiplies serialize the drain
        # (HW-measured +6 us with three late groups on Pool).
        pool_mult = {1, 3, 5}
    else:
        hh = h // 2 if h % 2 == 0 else h
        load_pieces = [(b, 0, h) for b in range(bs)]
        groups = [
            (b, i * hh, hh) for b in range(bs) for i in range(h // hh)
        ]
        pool_mult = {2}

    with tile.TileContext(nc) as tc, ExitStack() as ctx:
        consts = ctx.enter_context(tc.tile_pool(name="consts", bufs=1))
        xpool = ctx.enter_context(tc.tile_pool(name="xp", bufs=bs))
        opool = ctx.enter_context(tc.tile_pool(name="op", bufs=4))
        spool = ctx.enter_context(tc.tile_pool(name="sp", bufs=4))
        rpool = ctx.enter_context(tc.tile_pool(name="rp", bufs=4))
        psp = ctx.enter_context(tc.tile_pool(name="ps", bufs=4, space="PSUM"))

        # wh as bf16 [128, n_ch*128]: wh_bcast[p, j*128+m] = wh[j*128+p]
        # for all m - the contraction matmul then broadcasts s to all
        # 128 output partitions for free.
        wh_raw = consts.tile([P, n_ch], f32)
        nc.sync.dma_start(wh_raw[:], wh.rearrange("(j p) -> p j", p=P))
        wh_bcast = consts.tile([P, n_ch * P], bf16)
        nc.vector.tensor_copy(
            wh_bcast[:].rearrange("p (j m) -> p j m", j=n_ch),
            wh_raw[:, :, None].broadcast_to((P, n_ch, P)),
        )
        # W*bh replicated on all partitions, for the final bias add
        bh_sb = consts.tile([P, 1], f32)
        nc.sync.dma_start(bh_sb[:], bh.to_broadcast((P, 1)))
        biasW = consts.tile([P, 1], f32)
        nc.scalar.mul(biasW[:], bh_sb[:], float(w))

        # All loads first: SWDGE fp32 -> bf16 cast DMAs on the Pool queue.
        # (Pool's multiply work is emitted after every load is issued.)
        xbs = {}
        for b in range(bs):
            xbs[b] = xpool.tile([P, n_ch * hw], bf16, name=f"xb{b}", bufs=1)
        for b, r0, rn in load_pieces:
            xb = xbs[b]
            for ch in range(n_ch):
                nc.gpsimd.dma_start(
                    xb[:, ch * hw + r0 * w : ch * hw + (r0 + rn) * w],
                    x[b, ch * P : (ch + 1) * P, r0 : r0 + rn].rearrange(
                        "c h w -> c (h w)"
                    ),
                )

        # How many h-rows fit a 512-column matmul block
        rb = max(1, min(512 // w, h))

        for g, (b, h0, hr) in enumerate(groups):
            xb = xbs[b]
            xg = xb[:].rearrange("c (j h w) -> c j h w", j=n_ch, h=h)[
                :, :, h0 : h0 + hr
            ]

            # hx broadcast to all partitions: psum[m, (i,k)] = sum_c
            # wh[c] x[c, h0+i, k], accumulated over the two c-chunks,
            # one matmul pair per rb-row (512-col) block. Two psum tiles
            # per group (2 banks each) so the DVE reduce of the first
            # half overlaps the matmuls of the second - the PE is HAM
            # clock-throttled (K=4/8 for most of the run) so matmul
            # latency is the long pole of the per-group chain.
            rsg = rpool.tile([P, hr], f32)
            hs = max(rb, hr // 2)
            for p0 in range(0, hr, hs):
                pn = min(hs, hr - p0)
                pt = psp.tile([P, pn * w], f32)
                for q in range(0, pn, rb):
                    qn = min(rb, pn - q)
                    reg = pt[:, q * w : (q + qn) * w]
                    for ch in range(n_ch):
                        nc.tensor.matmul(
                            reg,
                            lhsT=wh_bcast[:, ch * P : (ch + 1) * P],
                            rhs=xg[:, ch, p0 + q : p0 + q + qn].rearrange(
                                "c h w -> c (h w)"
                            ),
                            start=(ch == 0),
                            stop=(ch == n_ch - 1),
                        )
                # s = row-sum(hx): full-lane PSUM -> SBUF reduce
                nc.vector.reduce_sum(
                    rsg[:, p0 : p0 + pn],
                    pt[:].rearrange("p (h w) -> p h w", w=w),
                    axis=X,
                )
            s128 = spool.tile([P, hr], f32)
            nc.scalar.add(s128[:], rsg[:], biasW[:])

            # o = s * x quantized to fp16, then store this group's rows
            ot = opool.tile([P, n_ch * hr * w], f16)
            eng = nc.gpsimd if g in pool_mult else nc.vector
            eng.tensor_mul(
                ot[:].rearrange("c (j h w) -> c j h w", j=n_ch, h=hr),
                xg,
                s128[:, None, :, None].broadcast_to((P, n_ch, hr, w)),
            )
            nc.scalar.dma_start(
                o[b, :, h0 : h0 + hr].rearrange("(j c) h w -> c j h w", c=P),
                ot[:].rearrange("c (j h w) -> c j h w", j=n_ch, h=hr),
            )
    _split_multi_waits(nc, mybir)
    return nc


def get_nc(bs=BS, c=C, h=H, w=W):
    key = (bs, c, h, w)
    if key not in _CACHE:
        _CACHE[key] = _build(bs, c, h, w)
    return _CACHE[key]


def kernel(x, wf, bf, wg, bg, wh, bh, **_unused):
    from concourse.bass_utils import run_bass_kernel_spmd

    x = np.ascontiguousarray(np.asarray(x, dtype=np.float32))
    wh = np.ascontiguousarray(np.asarray(wh, dtype=np.float32))
    bh = np.ascontiguousarray(np.asarray(bh, dtype=np.float32))

    in_maps = [
        {"x": x[k * BS : (k + 1) * BS], "wh": wh, "bh": bh} for k in range(N_CORES)
    ]
    # Tile scheduling is nondeterministic build-to-build and a rare schedule
    # can deadlock on hardware (NRT unrecoverable). Rebuilding produces a
    # fresh schedule, so retry with a clean build on any execution failure.
    last_err = None
    for attempt in range(3):
        try:
            nc = get_nc()
            res = run_bass_kernel_spmd(nc, in_maps, core_ids=list(range(N_CORES)))
            return np.concatenate(
                [
                    np.asarray(res.results[k]["o"], dtype=np.float32)
                    for k in range(N_CORES)
                ],
                axis=0,
            )
        except Exception as e:  # rebuild with a new schedule and retry
            last_err = e
            _CACHE.clear()
    raise last_err
